# revision 1
# baseline (speedup 1.0000x reference)
"""Bass/Trainium2 SPMD kernel for nn_Block3D (8 NeuronCores).

Sharding: spatial z-shard (24 planes -> 3 per core, host-prepped halo slabs),
channels on partitions. kernel_net GEMV1 row-sharded / GEMV2 K-sharded with a
single AllReduce; halo exchange of the LN2 output via paired AllGathers made
branch-free with per-core mask inputs. Depthwise 3x3x3 convs run as shifted-AP
scalar_tensor_tensor chains on DVE, with a tunable subset of MLP channel tiles
offloaded to the PE via host-precomputed diagonal weight matrices.
"""

import os
from contextlib import ExitStack

import numpy as np
import ml_dtypes

import concourse.bass as bass
import concourse.bacc as bacc
import concourse.tile as tile
from concourse import mybir
from concourse.bass_utils import run_bass_kernel_spmd

BF = ml_dtypes.bfloat16
F32 = mybir.dt.float32
BF16 = mybir.dt.bfloat16

C = 768
G = 12
GD = 64
S = 24
HID = 4 * C
KK = 27
V = S * S * S
EPS = 1e-5
NCORES = 8
ZP = S // NCORES
PL = S * S
VC = ZP * PL
Z5 = ZP + 2
PPL = 26 * 26
PADN = Z5 * PPL
CT = C // 128
HT = HID // 128
W1R = HID // NCORES
W2K = W1R
NB = 288
KFLAT = C * KK

PE_TILES = frozenset(int(x) for x in os.environ.get(
    "BLK3D_PE_TILES", ",".join(str(i) for i in range(24))).split(",") if x != "")

TAPS = [(dz, dy, dx) for dz in (-1, 0, 1) for dy in (-1, 0, 1) for dx in (-1, 0, 1)]

_CACHE = {}

Copy = mybir.ActivationFunctionType.Copy
Iden = mybir.ActivationFunctionType.Identity
Gelu = mybir.ActivationFunctionType.Gelu
Sigmoid = mybir.ActivationFunctionType.Sigmoid
Square = mybir.ActivationFunctionType.Square
Sqrt = mybir.ActivationFunctionType.Sqrt
Relu = mybir.ActivationFunctionType.Relu
ADD = mybir.AluOpType.add
SUB = mybir.AluOpType.subtract
MULT = mybir.AluOpType.mult

(S_Y, S_LB1, S_LB2, S_TPB, S_ABV, S_ABO, S_OPB, S_N2W, S_N2B, S_N3W, S_N3B,
 S_GNG, S_GNB, S_MODB) = range(14)


def _pad_off(dz, dy, dx):
    return (1 + dz) * PPL + (1 + dy) * 26 + (1 + dx)


def build_program():
    nc = bacc.Bacc("TRN2", target_bir_lowering=False)

    def dram_in(name, shape, dtype=F32):
        return nc.declare_dram_parameter(name, list(shape), dtype, isOutput=False)

    x_halo = dram_in("x_halo", [C, Z5 * PL], BF16)
    smalls = dram_in("smalls", [C, 16])
    knb1 = dram_in("knb1", [W1R])
    knb2r = dram_in("knb2r", [KFLAT])
    halo_masks = dram_in("halo_masks", [128, 18])
    gind6 = dram_in("gind6", [CT, 128, G])
    gexpT = dram_in("gexpT", [G, C])
    ident = dram_in("ident", [128, 128], BF16)
    onesc = dram_in("onesc", [128, 1], BF16)
    loraW1T = dram_in("loraW1T", [C, C], BF16)
    loraW2T = dram_in("loraW2T", [C, C], BF16)
    tpWT = dram_in("tpWT", [C, C], BF16)
    avWT = dram_in("avWT", [C, C], BF16)
    aoWT = dram_in("aoWT", [C, C], BF16)
    modWT = dram_in("modWT", [2 * C, C], BF16)
    w1sT = dram_in("w1sT", [2 * C, W1R], BF16)
    w2sT = dram_in("w2sT", [W2K, KFLAT], BF16)
    opT = dram_in("opT", [C, C], BF16)
    wiT = dram_in("wiT", [HT, 128, CT * 128], BF16)
    woT = dram_in("woT", [CT, 128, (HID // 2 // 128) * 128], BF16)
    dmlp = dram_in("dmlp", [HT, 128, KK * 128], BF16)
    dwk_in = dram_in("dwk", [HID, KK])
    out = nc.declare_dram_parameter("out", [C, VC], F32, isOutput=True)

    with tile.TileContext(nc) as tc, ExitStack() as ctx:
        dram = ctx.enter_context(tc.tile_pool(name="dram", bufs=1, space="DRAM"))
        persist = ctx.enter_context(tc.tile_pool(name="persist", bufs=1))
        gpool = ctx.enter_context(tc.tile_pool(name="gemv", bufs=2))
        wpool = ctx.enter_context(tc.tile_pool(name="wstream", bufs=4))

        # ------------- persistent small tiles -------------
        sm = [persist.tile([128, 16], F32, name=f"sm{i}", tag=f"sm{i}")
              for i in range(CT)]
        for i in range(CT):
            nc.gpsimd.dma_start(sm[i][:], smalls[128 * i:128 * (i + 1), :])
        hm = persist.tile([128, 18], F32, name="hm", tag="hm")
        nc.gpsimd.dma_start(hm[:], halo_masks[:, :])
        id_t = persist.tile([128, 128], BF16, name="identt", tag="identt")
        nc.gpsimd.dma_start(id_t[:], ident[:, :])
        ones_t = persist.tile([128, 1], BF16, name="onest", tag="onest")
        nc.gpsimd.dma_start(ones_t[:], onesc[:, :])
        eps_t = persist.tile([128, 1], F32, name="epst", tag="epst")
        nc.vector.memset(eps_t[:], EPS)
        mid = {}
        for k in range(16):
            t = persist.tile([128, 128], BF16, name=f"mid{k}", tag=f"mid{k}")
            nc.vector.tensor_scalar_mul(t[:], id_t[:], hm[:, k:k + 1])
            mid[k] = t
        junk = persist.tile([128, VC], BF16, name="junk", tag="junk")
        _dwk = []
        for tt in range(HT):
            t = persist.tile([128, KK], F32, name=f"dwk{tt}", tag=f"dwk{tt}")
            nc.gpsimd.dma_start(t[:], dwk_in[128 * tt:128 * (tt + 1), :])
            _dwk.append(t)

        def conv_dve(dst, src_pad, src_pad1, ktile):
            p4 = src_pad.rearrange("p (z y x) -> p z y x", z=Z5, y=26, x=26)
            p4s = src_pad1.rearrange("p (z y x) -> p z y x", z=Z5, y=26, x=26)
            d4 = dst.rearrange("p (z y x) -> p z y x", z=ZP, y=S, x=S)
            for ti, (dz, dy, dx) in enumerate(TAPS):
                sc = ktile[:, ti:ti + 1]
                for z in range(ZP):
                    if _pad_off(dz, dy, dx) % 2 == 0:
                        sv = p4[:, 1 + dz + z, 1 + dy:1 + dy + S,
                                1 + dx:1 + dx + S]
                    else:
                        sv = p4s[:, 1 + dz + z, 1 + dy:1 + dy + S, dx:dx + S]
                    dv = d4[:, z]
                    if ti == 0:
                        nc.vector.tensor_scalar_mul(dv, sv, sc)
                    else:
                        nc.vector.scalar_tensor_tensor(dv, sv, sc, dv,
                                                       op0=MULT, op1=ADD)

        def ln_stats(tiles, tag):
            """Per-voxel mean/rstd over channels -> [128, VC] bf16 bcast tiles."""
            chunks = [(0, 512), (512, 512), (1024, 512), (1536, 192)]
            with (tc.tile_pool(name=f"{tag}ps", bufs=1, space="PSUM") as lps,
                  tc.tile_pool(name=f"{tag}sq", bufs=2) as sqp,
                  tc.tile_pool(name=f"{tag}rw", bufs=1) as rwp):
                ps1 = lps.tile([1, VC], F32, name="s1", tag="s1")
                ps2 = lps.tile([1, VC], F32, name="s2", tag="s2")
                for o, n in chunks:
                    sl = slice(o, o + n)
                    for k in range(CT):
                        nc.tensor.matmul(ps1[:, sl], ones_t[:], tiles[k][:, sl],
                                         start=(k == 0), stop=(k == CT - 1))
                    for k in range(CT):
                        sq = sqp.tile([128, 512], BF16, name="sq", tag="sq")
                        nc.scalar.activation(sq[:, 0:n], tiles[k][:, sl], Square)
                        nc.tensor.matmul(ps2[:, sl], ones_t[:], sq[:, 0:n],
                                         start=(k == 0), stop=(k == CT - 1))
                row = rwp.tile([1, 2 * VC], F32, name="row", tag="row")
                nc.scalar.activation(row[:, 0:VC], ps1[:], Copy, scale=1.0 / C)
                nc.scalar.activation(row[:, VC:2 * VC], ps2[:], Copy,
                                     scale=1.0 / C)
                w = VC // 96  # 18
                rs = rwp.tile([96, 2 * w], F32, name="rs", tag="rs")
                nc.gpsimd.dma_start(rs[:, 0:w], row[:, 0:VC])
                nc.gpsimd.dma_start(rs[:, w:2 * w], row[:, VC:2 * VC])
                m2 = rwp.tile([96, w], F32, name="m2", tag="m2")
                nc.scalar.square(m2[:], rs[:, 0:w])
                vr = rwp.tile([96, w], F32, name="vr", tag="vr")
                nc.vector.tensor_sub(vr[:], rs[:, w:2 * w], m2[:])
                nc.scalar.activation(vr[:], vr[:], Sqrt, bias=eps_t[0:96, 0:1])
                nc.vector.reciprocal(vr[:], vr[:])
                mrow = rwp.tile([1, 2 * VC], BF16, name="mrow", tag="mrow")
                nc.gpsimd.dma_start(mrow[:, 0:VC], rs[:, 0:w])
                nc.gpsimd.dma_start(mrow[:, VC:2 * VC], vr[:])
                drow = dram.tile([2 * VC], BF16, name=f"{tag}drow",
                                 tag=f"{tag}drow")
                nc.gpsimd.dma_start(drow[:], mrow[:])
            muB = persist.tile([128, VC], BF16, name=f"{tag}muB", tag=f"{tag}muB")
            rsB = persist.tile([128, VC], BF16, name=f"{tag}rsB", tag=f"{tag}rsB")
            nc.gpsimd.dma_start(
                muB[:], bass.AP(tensor=drow[:].tensor, offset=drow[:].offset,
                                ap=[[0, 128], [1, VC]]))
            nc.gpsimd.dma_start(
                rsB[:], bass.AP(tensor=drow[:].tensor,
                                offset=drow[:].offset + VC,
                                ap=[[0, 128], [1, VC]]))
            return muB, rsB

        xbp = ctx.enter_context(tc.tile_pool(name="xbp", bufs=1))
        gatep = ctx.enter_context(tc.tile_pool(name="gatep", bufs=1))
        xb = [xbp.tile([128, VC], BF16, name=f"xb{i}", tag=f"xb{i}")
              for i in range(CT)]
        gate = [gatep.tile([128, VC], BF16, name=f"gate{j}", tag=f"gate{j}")
                for j in range(HT // 2)]

        with tc.tile_pool(name="actp", bufs=1) as actp:
            xs = [actp.tile([128, Z5 * PL], BF16, name=f"xs{i}", tag=f"xs{i}")
                  for i in range(CT)]
            for i in range(CT):
                nc.gpsimd.dma_start(xs[i][:], x_halo[128 * i:128 * (i + 1), :])
            dyn = [actp.tile([128, VC], BF16, name=f"dyn{i}", tag=f"dyn{i}")
                   for i in range(CT)]

            # ---- phase A: vc partial sums + AR1 ----
            vcs = persist.tile([128, CT], F32, name="vcs", tag="vcs")
            for i in range(CT):
                nc.scalar.activation(junk[:], xs[i][:, PL:PL + VC], Copy,
                                     accum_out=vcs[:, i:i + 1])
            ar1_in = dram.tile([C], F32, name="ar1i", tag="ar1i")
            ar1_out = dram.tile([C], F32, name="ar1o", tag="ar1o", addr_space="Shared")
            nc.gpsimd.dma_start(
                bass.AP(tensor=ar1_in[:].tensor, offset=ar1_in[:].offset,
                        ap=[[1, 128], [128, CT]]), vcs[:])
            nc.gpsimd.collective_compute(
                "AllReduce", ADD, replica_groups=[list(range(NCORES))],
                ins=[ar1_in[:]], outs=[ar1_out[:]])

            # ---- phases B-E ----
            with tc.tile_pool(name="psA", bufs=2, space="PSUM") as psA:

                def load_w(wdram, k0, m0, tag):
                    t = wpool.tile([128, 128], BF16, name=tag, tag=tag)
                    nc.gpsimd.dma_start(t[:], wdram[k0:k0 + 128, m0:m0 + 128])
                    return t

                def gemv(wdram, in_cols, nk, nm, act, bias_cols, tag, scale=1.0,
                         odt=BF16):
                    outs = []
                    for m in range(nm):
                        ps = psA.tile([128, 1], F32, name="ps_small",
                                      tag="ps_small")
                        for k in range(nk):
                            w = load_w(wdram, 128 * k, 128 * m, tag)
                            nc.tensor.matmul(ps[:], w[:], in_cols[k][:],
                                             start=(k == 0), stop=(k == nk - 1))
                        o = gpool.tile([128, 1], odt, name=f"{tag}o{m}",
                                       tag=f"{tag}o{m}")
                        bias = bias_cols[m] if bias_cols is not None else 0.0
                        nc.scalar.activation(o[:], ps[:], act, bias=bias,
                                             scale=scale)
                        outs.append(o)
                    return outs

                t_cols = []
                for i in range(CT):
                    t = gpool.tile([128, 1], BF16, name=f"tc{i}", tag=f"tc{i}")
                    nc.scalar.activation(t[:], sm[i][:, S_Y:S_Y + 1], Copy)
                    t_cols.append(t)
                h1 = gemv(loraW1T, t_cols, CT, CT, Relu,
                          [sm[i][:, S_LB1:S_LB1 + 1] for i in range(CT)], "lw1")
                h2 = gemv(loraW2T, h1, CT, CT, Iden,
                          [sm[i][:, S_LB2:S_LB2 + 1] for i in range(CT)], "lw2")
                tp = gemv(tpWT, h2, CT, CT, Iden,
                          [sm[i][:, S_TPB:S_TPB + 1] for i in range(CT)], "tpw")
                av = gemv(avWT, tp, CT, CT, Iden,
                          [sm[i][:, S_ABV:S_ABV + 1] for i in range(CT)], "avw")
                attn = gemv(aoWT, av, CT, CT, Iden,
                            [sm[i][:, S_ABO:S_ABO + 1] for i in range(CT)],
                            "aow")

                comb = []
                for i in range(CT):
                    cb = gpool.tile([128, 1], BF16, name=f"cmb{i}",
                                    tag=f"cmb{i}")
                    col = persist.tile([128, 1], F32, name=f"vcc{i}",
                                       tag=f"vcc{i}")
                    nc.gpsimd.dma_start(
                        col[:], bass.AP(tensor=ar1_out[:].tensor,
                                        offset=ar1_out[:].offset + 128 * i,
                                        ap=[[1, 128], [128, 1]]))
                    nc.scalar.activation(cb[:], col[:], Copy, scale=1.0 / V)
                    comb.append(cb)
                comb += attn

                mod = gemv(modWT, comb, 2 * CT, CT, Sigmoid,
                           [sm[i][:, S_MODB:S_MODB + 1] for i in range(CT)],
                           "modw", odt=F32)

                knb1_t = persist.tile([128, 3], F32, name="knb1t", tag="knb1t")
                nc.gpsimd.dma_start(
                    knb1_t[:],
                    bass.AP(tensor=knb1, offset=0, ap=[[1, 128], [128, 3]]))
                kp1 = gemv(w1sT, comb, 2 * CT, 3, Relu,
                           [knb1_t[:, m:m + 1] for m in range(3)], "w1s")

                ar2_in = dram.tile([KFLAT], F32, name="ar2i", tag="ar2i")
                ar2_out = dram.tile([KFLAT], F32, name="ar2o", tag="ar2o", addr_space="Shared")
                with tc.tile_pool(name="kseq", bufs=2) as kseq:
                    for t in range(KK):
                        wts = []
                        for k in range(3):
                            wt = kseq.tile([128, C], BF16, name=f"w2s{k}",
                                           tag=f"w2s{k}")
                            nc.sync.dma_start(
                                wt[:], w2sT[128 * k:128 * (k + 1),
                                            C * t:C * (t + 1)])
                            wts.append(wt)
                        ps = psA.tile([128, CT], F32, name="g2ps", tag="g2ps")
                        for m in range(CT):
                            for k in range(3):
                                nc.tensor.matmul(
                                    ps[:, m:m + 1],
                                    wts[k][:, 128 * m:128 * (m + 1)],
                                    kp1[k][:], start=(m == 0 and k == 0),
                                    stop=(m == CT - 1 and k == 2))
                        ko = kseq.tile([128, CT], F32, name="g2o", tag="g2o")
                        nc.scalar.activation(ko[:], ps[:], Copy)
                        for m in range(CT):
                            nc.gpsimd.dma_start(
                                bass.AP(tensor=ar2_in[:].tensor,
                                        offset=(ar2_in[:].offset + C * t
                                                + 128 * m),
                                        ap=[[1, 128], [1, 1]]),
                                ko[:, m:m + 1])
                nc.gpsimd.collective_compute(
                    "AllReduce", ADD, replica_groups=[list(range(NCORES))],
                    ins=[ar2_in[:]], outs=[ar2_out[:]])

                kern = []
                for i in range(CT):
                    kt = persist.tile([128, KK], F32, name=f"kern{i}",
                                      tag=f"kern{i}")
                    nc.gpsimd.dma_start(
                        kt[:], bass.AP(tensor=ar2_out[:].tensor,
                                       offset=ar2_out[:].offset + 128 * i,
                                       ap=[[1, 128], [C, KK]]))
                    kb = persist.tile([128, KK], F32, name=f"kernb{i}",
                                      tag=f"kernb{i}")
                    nc.gpsimd.dma_start(
                        kb[:], bass.AP(tensor=knb2r, offset=128 * i,
                                       ap=[[1, 128], [C, KK]]))
                    nc.vector.tensor_add(kt[:], kt[:], kb[:])
                    kern.append(kt)

                # ---- phase E: mv + dynamic conv + GN stats ----
                ar3_in = dram.tile([G, 2], F32, name="ar3i", tag="ar3i")
                ar3_out = dram.tile([G, 2], F32, name="ar3o", tag="ar3o", addr_space="Shared")
                gsb = persist.tile([G, 2], F32, name="gsb", tag="gsb")
                with (tc.tile_pool(name="mvp", bufs=2) as mvp,
                      tc.tile_pool(name="gnps", bufs=1, space="PSUM") as gnps):
                    gps = gnps.tile([G, 2], F32, name="gps", tag="gps")
                    for i in range(CT):
                        mp = mvp.tile([128, PADN], BF16, name="mvpad",
                                      tag="mvpad")
                        mp1 = mvp.tile([128, PADN], BF16, name="mvpad1",
                                       tag="mvpad1")
                        nc.gpsimd.memset(mp[:], 0.0)
                        nc.gpsimd.memset(mp1[:], 0.0)
                        m4 = mp.rearrange("p (z y x) -> p z y x", z=Z5, y=26,
                                          x=26)
                        x4 = xs[i].rearrange("p (z y x) -> p z y x", z=Z5, y=S,
                                             x=S)
                        for z in range(Z5):
                            nc.scalar.activation(m4[:, z, 1:25, 1:25], x4[:, z],
                                                 Copy, scale=mod[i][:, 0:1])
                        nc.scalar.activation(mp1[:, 0:PADN - 1], mp[:, 1:PADN],
                                             Copy)
                        conv_dve(dyn[i][:], mp[:], mp1[:], kern[i])
                        st = mvp.tile([128, 2], F32, name="gnst", tag="gnst")
                        nc.scalar.activation(junk[:], dyn[i][:], Copy,
                                             accum_out=st[:, 0:1])
                        nc.scalar.activation(junk[:], dyn[i][:], Square,
                                             accum_out=st[:, 1:2])
                        gi = mvp.tile([128, G], F32, name="gind", tag="gind")
                        nc.gpsimd.dma_start(gi[:], gind6[i, :, :])
                        nc.tensor.matmul(gps[:], gi[:], st[:], start=(i == 0),
                                         stop=(i == CT - 1))
                    nc.scalar.activation(gsb[:], gps[:], Copy)
                nc.gpsimd.dma_start(ar3_in[:], gsb[:])
                nc.gpsimd.collective_compute(
                    "AllReduce", ADD, replica_groups=[list(range(NCORES))],
                    ins=[ar3_in[:]], outs=[ar3_out[:]])
                gstat = persist.tile([G, 2], F32, name="gstat", tag="gstat")
                nc.gpsimd.dma_start(gstat[:], ar3_out[:])
                NGRP = float(GD * V)
                gmr = persist.tile([G, 2], F32, name="gmr", tag="gmr")
                nc.scalar.activation(gmr[:, 0:1], gstat[:, 0:1], Copy,
                                     scale=1.0 / NGRP)
                musq = persist.tile([G, 1], F32, name="musq", tag="musq")
                nc.scalar.square(musq[:], gmr[:, 0:1])
                var = persist.tile([G, 1], F32, name="gvar", tag="gvar")
                nc.vector.tensor_scalar(var[:], gstat[:, 1:2], 1.0 / NGRP, None,
                                        op0=MULT)
                nc.vector.tensor_sub(var[:], var[:], musq[:])
                nc.scalar.activation(var[:], var[:], Sqrt, bias=eps_t[0:G, 0:1])
                nc.vector.reciprocal(gmr[:, 1:2], var[:])

                opT_t = [persist.tile([128, C], BF16, name=f"opT{i}",
                                      tag=f"opT{i}") for i in range(CT)]
                for i in range(CT):
                    nc.sync.dma_start(opT_t[i][:],
                                        opT[128 * i:128 * (i + 1), :])
                cafm_shift = []
                gsc = []
                for i in range(CT):
                    ge = gpool.tile([G, 128], F32, name=f"gexp{i}",
                                    tag=f"gexp{i}")
                    nc.gpsimd.dma_start(ge[:], gexpT[:, 128 * i:128 * (i + 1)])
                    ps = psA.tile([128, 2], F32, name="ps_sm2", tag="ps_sm2")
                    nc.tensor.matmul(ps[:], ge[:], gmr[:], start=True, stop=True)
                    mu_c = persist.tile([128, 2], F32, name=f"muc{i}",
                                        tag=f"muc{i}")
                    nc.scalar.activation(mu_c[:], ps[:], Copy)
                    a = persist.tile([128, 1], F32, name=f"gsc{i}",
                                     tag=f"gsc{i}")
                    nc.vector.tensor_mul(a[:], sm[i][:, S_GNG:S_GNG + 1],
                                         mu_c[:, 1:2])
                    b = persist.tile([128, 1], F32, name=f"gsh{i}",
                                     tag=f"gsh{i}")
                    nc.vector.tensor_mul(b[:], mu_c[:, 0:1], a[:])
                    nc.vector.tensor_sub(b[:], sm[i][:, S_GNB:S_GNB + 1], b[:])
                    gsc.append(a)
                    bb = gpool.tile([128, 1], BF16, name=f"gshb{i}",
                                    tag=f"gshb{i}")
                    nc.scalar.activation(bb[:], b[:], Copy)
                    cafm_shift.append(bb)
                cb_cols = []
                for m in range(CT):
                    ps = psA.tile([128, 1], F32, name="ps_small",
                                  tag="ps_small")
                    for k in range(CT):
                        nc.tensor.matmul(ps[:],
                                         opT_t[k][:, 128 * m:128 * (m + 1)],
                                         cafm_shift[k][:], start=(k == 0),
                                         stop=(k == CT - 1))
                    o = persist.tile([128, 1], F32, name=f"cbc{m}",
                                     tag=f"cbc{m}")
                    nc.scalar.activation(o[:], ps[:], Iden,
                                         bias=sm[m][:, S_OPB:S_OPB + 1])
                    cb_cols.append(o)
                for i in range(CT):
                    nc.vector.tensor_scalar_mul(opT_t[i][:], opT_t[i][:],
                                                gsc[i][:])

            # ---- phase F1: cafm matmul + xb ----
            CH4 = [(0, 512), (512, 512), (1024, 512), (1536, 192)]
            with tc.tile_pool(name="opwps", bufs=2, space="PSUM") as opwps:
                for m in range(CT):
                    for o, n in CH4:
                        sl = slice(o, o + n)
                        ps = opwps.tile([128, 512], F32, name="opw", tag="opw")
                        for k in range(CT):
                            nc.tensor.matmul(
                                ps[:, 0:n], opT_t[k][:, 128 * m:128 * (m + 1)],
                                dyn[k][:, sl], start=(k == 0),
                                stop=(k == CT - 1))
                        nc.vector.scalar_tensor_tensor(
                            xb[m][:, sl], ps[:, 0:n], cb_cols[m][:],
                            xs[m][:, PL + o:PL + o + n],
                            op0=ADD, op1=MULT)

        # ---- phase F2: LN2 + halo exchange ----
        muB, rsB = ln_stats(xb, "ln2")
        with tc.tile_pool(name="xlnp", bufs=1) as xlnp:
            xln = [xlnp.tile([128, Z5 * PL], BF16, name=f"xln{i}",
                             tag=f"xln{i}") for i in range(CT)]
            with tc.tile_pool(name="glueF", bufs=2) as glueF:
                for i in range(CT):
                    t1 = glueF.tile([128, VC], BF16, name="lnt1", tag="lnt1")
                    nc.vector.tensor_sub(t1[:], xb[i][:], muB[:])
                    nc.vector.tensor_mul(t1[:], t1[:], rsB[:])
                    nc.vector.tensor_scalar(xln[i][:, PL:PL + VC], t1[:],
                                            sm[i][:, S_N2W:S_N2W + 1],
                                            sm[i][:, S_N2B:S_N2B + 1],
                                            op0=MULT, op1=ADD)

            agi = dram.tile([2, C, PL], BF16, name="agi", tag="agi")
            ago = dram.tile([8, 2, C, PL], BF16, name="ago", tag="ago", addr_space="Shared")
            for i in range(CT):
                cs = slice(128 * i, 128 * (i + 1))
                nc.gpsimd.dma_start(agi[0, cs, :], xln[i][:, PL:2 * PL])
                nc.gpsimd.dma_start(agi[1, cs, :], xln[i][:, 3 * PL:4 * PL])
            nc.gpsimd.collective_compute(
                "AllGather", mybir.AluOpType.bypass,
                replica_groups=[list(range(NCORES))],
                ins=[agi[:]], outs=[ago[:]])
            with (tc.tile_pool(name="halo_ps", bufs=2, space="PSUM") as hps,
                  tc.tile_pool(name="hterm", bufs=2) as htp):
                for i in range(CT):
                    cs = slice(128 * i, 128 * (i + 1))
                    # lower halo <- senders' top face (idx 1), masks 0..7;
                    # upper halo <- senders' bottom face (idx 0), masks 8..15
                    for face, dst0, m0 in [(1, 0, 0), (0, 4 * PL, 8)]:
                        terms = []
                        for j in range(NCORES):
                            t = htp.tile([128, PL], BF16, name=f"ht{j}",
                                         tag=f"ht{j}")
                            nc.sync.dma_start(t[:], ago[j, face, cs, :])
                            terms.append((m0 + j, t))
                        for nb in range(PL // NB):
                            sl = slice(NB * nb, NB * (nb + 1))
                            ps = hps.tile([128, NB], F32, name="hh", tag="hh")
                            for ti, (mc, t) in enumerate(terms):
                                nc.tensor.matmul(ps[:], mid[mc][:], t[:, sl],
                                                 start=(ti == 0),
                                                 stop=(ti == NCORES - 1))
                            nc.scalar.activation(
                                xln[i][:, dst0 + NB * nb:dst0 + NB * (nb + 1)],
                                ps[:], Copy)

            # ---- phase G: MLP ----
            with (tc.tile_pool(name="hpadp", bufs=2) as hpad_pool,
                  tc.tile_pool(name="hpad1p", bufs=1) as hpad1_pool,
                  tc.tile_pool(name="wiw", bufs=2) as wiw,
                  tc.tile_pool(name="diag", bufs=1) as dpool,
                  tc.tile_pool(name="glueG", bufs=1) as glueG,
                  tc.tile_pool(name="wips", bufs=2, space="PSUM") as wips,
                  tc.tile_pool(name="cvps", bufs=1, space="PSUM") as cvps):

                def mlp_tile(tt, conv_out):
                    wall = wiw.tile([128, CT * 128], BF16, name="wiall",
                                    tag="wiall")
                    nc.sync.dma_start(wall[:], wiT[tt, :, :])
                    wts = [wall[:, 128 * k:128 * (k + 1)] for k in range(CT)]
                    hp = hpad_pool.tile([128, PADN], BF16, name="hpad",
                                        tag="hpad")
                    nc.gpsimd.memset(hp[:], 0.0)
                    h4 = hp.rearrange("p (z y x) -> p z y x", z=Z5, y=26, x=26)
                    for nb in range(Z5 * PL // NB):
                        sl = slice(NB * nb, NB * (nb + 1))
                        ps = wips.tile([128, NB], F32, name="wi_ps",
                                       tag="wi_ps")
                        for k in range(CT):
                            nc.tensor.matmul(ps[:], wts[k], xln[k][:, sl],
                                             start=(k == 0), stop=(k == CT - 1))
                        z, y0 = nb // 2, (nb % 2) * 12
                        nc.scalar.activation(h4[:, z, 1 + y0:13 + y0, 1:25],
                                             ps[:], Copy)
                    if tt in PE_TILES:
                        p4 = hp.rearrange("p (z y x) -> p z y x", z=Z5, y=26,
                                          x=26)
                        cps = [cvps.tile([128, NB], F32, name=f"cv{nb}",
                                         tag=f"cv{nb}")
                               for nb in range(VC // NB)]
                        dga = dpool.tile([128, KK * 128], BF16, name="dgall",
                                         tag="dgall")
                        nc.sync.dma_start(dga[:], dmlp[tt, :, :])
                        for ti, (dz, dy, dx) in enumerate(TAPS):
                            dg = dga[:, 128 * ti:128 * (ti + 1)]
                            for nb in range(VC // NB):
                                z, y0 = nb // 2, (nb % 2) * 12
                                sv = p4[:, 1 + z + dz,
                                        1 + y0 + dy:13 + y0 + dy,
                                        1 + dx:25 + dx]
                                nc.tensor.matmul(cps[nb][:], dg, sv,
                                                 start=(ti == 0),
                                                 stop=(ti == KK - 1))
                        return cps
                    hp1 = hpad1_pool.tile([128, PADN], BF16, name="hpad1",
                                          tag="hpad1")
                    nc.gpsimd.memset(hp1[:], 0.0)
                    nc.scalar.activation(hp1[:, 0:PADN - 1], hp[:, 1:PADN],
                                         Copy)
                    conv_dve(conv_out[:], hp[:], hp1[:], _dwk[tt])
                    return None

                for j in range(HT // 2):
                    c1 = glueG.tile([128, VC], BF16, name="conv1", tag="conv1")
                    p1 = mlp_tile(j, c1)
                    g1 = glueG.tile([128, VC], BF16, name="gelu1", tag="gelu1")
                    if p1 is not None:
                        for nb in range(VC // NB):
                            nc.scalar.activation(g1[:, NB * nb:NB * (nb + 1)],
                                                 p1[nb][:], Gelu)
                    else:
                        nc.scalar.activation(g1[:], c1[:], Gelu)
                    c2 = glueG.tile([128, VC], BF16, name="conv2", tag="conv2")
                    p2 = mlp_tile(j + HT // 2, c2)
                    if p2 is not None:
                        for nb in range(VC // NB):
                            nc.scalar.activation(c2[:, NB * nb:NB * (nb + 1)],
                                                 p2[nb][:], Copy)
                    nc.vector.tensor_mul(gate[j][:], g1[:], c2[:])

        # ---- phase H: Wo + residual ----
        ytp = ctx.enter_context(tc.tile_pool(name="ytp", bufs=1))
        y_t = [ytp.tile([128, VC], BF16, name=f"y{i}", tag=f"y{i}")
               for i in range(CT)]
        with (tc.tile_pool(name="wow", bufs=2) as wow,
              tc.tile_pool(name="wops", bufs=2, space="PSUM") as wops):
            for m in range(CT):
                wall = wow.tile([128, (HT // 2) * 128], BF16, name="woall",
                                tag="woall")
                nc.sync.dma_start(wall[:], woT[m, :, :])
                wts = [wall[:, 128 * k:128 * (k + 1)] for k in range(HT // 2)]
                for o, n in [(0, 512), (512, 512), (1024, 512), (1536, 192)]:
                    sl = slice(o, o + n)
                    ps = wops.tile([128, 512], F32, name="wo_ps", tag="wo_ps")
                    for k in range(HT // 2):
                        nc.tensor.matmul(ps[:, 0:n], wts[k], gate[k][:, sl],
                                         start=(k == 0), stop=False)
                    nc.tensor.matmul(ps[:, 0:n], id_t[:], xb[m][:, sl],
                                     start=False, stop=True)
                    nc.scalar.activation(y_t[m][:, sl], ps[:, 0:n], Copy)

        # ---- LN3 + output ----
        muB3, rsB3 = ln_stats(y_t, "ln3")
        with tc.tile_pool(name="glueH", bufs=2) as glueH:
            for i in range(CT):
                t1 = glueH.tile([128, VC], BF16, name="ln3t", tag="ln3t")
                nc.vector.tensor_sub(t1[:], y_t[i][:], muB3[:])
                nc.vector.tensor_mul(t1[:], t1[:], rsB3[:])
                of = glueH.tile([128, VC], F32, name="outf", tag="outf")
                nc.vector.tensor_scalar(of[:], t1[:],
                                        sm[i][:, S_N3W:S_N3W + 1],
                                        sm[i][:, S_N3B:S_N3B + 1],
                                        op0=MULT, op1=ADD)
                nc.gpsimd.dma_start(out[128 * i:128 * (i + 1), :], of[:])

    nc.compile()
    return nc


def _prep(inputs):
    bf = lambda a: np.ascontiguousarray(a).astype(BF)
    f32 = lambda a: np.ascontiguousarray(a, dtype=np.float32)
    x = f32(inputs["x"][0])
    xf = x.reshape(C, S, PL)

    smalls = np.zeros((C, 16), np.float32)
    smalls[:, 0] = f32(inputs["y"][0, 0])
    for i, k in enumerate(["lora_b1", "lora_b2", "tp_b", "attn_bv", "attn_bo",
                           "op_b", "n2_w", "n2_b", "n3_w", "n3_b", "gn_g",
                           "gn_b", "mod_b"]):
        smalls[:, i + 1] = f32(inputs[k])

    gind6 = np.zeros((CT, 128, G), np.float32)
    for j in range(CT):
        for p in range(128):
            gind6[j, p, (128 * j + p) // GD] = 1.0
    gexpT = np.zeros((G, C), np.float32)
    for c in range(C):
        gexpT[c // GD, c] = 1.0

    kn_W2 = f32(inputs["kn_W2"])
    w2r = kn_W2.reshape(C, KK, HID).transpose(1, 0, 2).reshape(KFLAT, HID)
    w2T = np.ascontiguousarray(w2r.T)
    knb2r = f32(inputs["kn_b2"]).reshape(C, KK).T.copy().reshape(-1)
    kn_W1 = f32(inputs["kn_W1"])

    mlp_dw = f32(inputs["mlp_dw"]).reshape(HID, KK)
    dmlp = np.zeros((HT, 128, KK, 128), np.float32)
    idx = np.arange(128)
    for tt in range(HT):
        for ti in range(KK):
            dmlp[tt, idx, ti, idx] = mlp_dw[128 * tt:128 * (tt + 1), ti]
    dmlp = dmlp.reshape(HT, 128, KK * 128)

    com = dict(
        smalls=smalls, knb2r=knb2r,
        gind6=gind6, gexpT=gexpT,
        ident=bf(np.eye(128, dtype=np.float32)),
        onesc=bf(np.ones((128, 1), np.float32)),
        loraW1T=bf(f32(inputs["lora_W1"]).T),
        loraW2T=bf(f32(inputs["lora_W2"]).T),
        tpWT=bf(f32(inputs["tp_W"]).T), avWT=bf(f32(inputs["attn_Wv"]).T),
        aoWT=bf(f32(inputs["attn_Wo"]).T), modWT=bf(f32(inputs["mod_W"]).T),
        opT=bf(f32(inputs["op_W"]).T),
        wiT=bf(f32(inputs["mlp_Wi"]).T.reshape(CT, 128, HT, 128)
               .transpose(2, 1, 0, 3).reshape(HT, 128, CT * 128)),
        woT=bf(f32(inputs["mlp_Wo"]).T.reshape(HT // 2, 128, CT, 128)
               .transpose(2, 1, 0, 3).reshape(CT, 128, (HT // 2) * 128)),
        dmlp=bf(dmlp), dwk=mlp_dw,
    )

    in_maps = []
    for i in range(NCORES):
        z0 = ZP * i
        xh = np.zeros((C, Z5, PL), np.float32)
        lo, hi = max(z0 - 1, 0), min(z0 + ZP + 1, S)
        xh[:, lo - (z0 - 1):lo - (z0 - 1) + (hi - lo)] = xf[:, lo:hi]
        hmask = np.zeros((128, 18), np.float32)
        if i > 0:
            hmask[:, i - 1] = 1.0
        if i < NCORES - 1:
            hmask[:, 8 + i + 1] = 1.0
        m = dict(com)
        m.update(
            x_halo=xh.reshape(C, Z5 * PL).astype(BF),
            knb1=f32(inputs["kn_b1"][W1R * i:W1R * (i + 1)]),
            halo_masks=hmask,
            w1sT=bf(kn_W1[W1R * i:W1R * (i + 1), :].T),
            w2sT=bf(w2T[W2K * i:W2K * (i + 1), :]),
        )
        in_maps.append(m)
    return in_maps


def kernel(**inputs) -> np.ndarray:
    if "nc" not in _CACHE:
        _CACHE["nc"] = build_program()
    nc = _CACHE["nc"]
    in_maps = _prep(inputs)
    res = run_bass_kernel_spmd(nc, in_maps, list(range(NCORES)))
    outs = [res.results[i]["out"].reshape(C, ZP, PL) for i in range(NCORES)]
    full = np.concatenate(outs, axis=1)
    return full.reshape(1, C, S, S, S).astype(np.float32)

